# revision 11
# baseline (speedup 1.0000x reference)
"""Causal single-head attention (B=8, S=2048, D=2048, H=128) on 8 TRN2 NeuronCores.

Strategy: data-parallel over batch — core b computes batch element b entirely
on-chip; no collectives. Per core:

  - x [S, D] f32 is cast-DMA'd to bf16; x^T chunks [128d, 512s] are produced
    by PE transposes grouped 4-to-a-PSUM-bank with one wide DVE copy each.
  - Q^T, K^T, V^T [h, s] via matmuls with Wq/Wk/Wv chunks stationary (bf16,
    1 cycle/row, moving free 512); V rechunked to [k, h] by PE transposes.
  - scores^T [k, q] = (lhsT=K^T chunk).T @ Q^T slice; exp on ScalarE with the
    1/sqrt(H) scale folded in; causal: upper-triangle chunks skipped, diagonal
    chunks zeroed post-exp via gpsimd affine_select.
  - AV runs L-behind the scores/exp pipeline (sliding window) so exp latency
    never stalls the PE; softmax denominators accumulate on GpSimd in f32,
    then one ones-matmul per q-block + tiny outer-product transposes give the
    per-q reciprocals; epilogue transposes run in bf16.

All matmuls bf16 (f32 PSUM accumulation); rel err vs the f32 reference ~5e-3.
"""

import numpy as np

import concourse.bass as bass
import concourse.mybir as mybir
import concourse.tile as tile
from concourse import bacc
from concourse.bass_utils import run_bass_kernel_spmd
from concourse.masks import make_identity

B, S, D, H = 8, 2048, 2048, 128
P = 128
DC = D // P            # 16 d-chunks (contraction)
SC = S // P            # 16 s-chunks
QB = 512               # q-block (moving free dim)
NQ = S // QB           # 4 q-blocks
SPB = QB // P          # 4 s-chunks per q-block
SCALE = float(H) ** -0.5
AV_LAG = 4             # AV trails scores/exp by this many k-chunks

F32 = mybir.dt.float32
BF16 = mybir.dt.bfloat16

_NC_CACHE = None


def build():
    nc = bacc.Bacc(None, target_bir_lowering=False)

    x_d = nc.declare_dram_parameter("x", [S, D], F32, isOutput=False)
    wq_d = nc.declare_dram_parameter("Wq", [D, H], F32, isOutput=False)
    wk_d = nc.declare_dram_parameter("Wk", [D, H], F32, isOutput=False)
    wv_d = nc.declare_dram_parameter("Wv", [D, H], F32, isOutput=False)
    out_d = nc.declare_dram_parameter("out", [S, H], F32, isOutput=True)

    with tile.TileContext(nc) as tc:
        with (
            tc.tile_pool(name="const", bufs=1) as const,
            tc.tile_pool(name="persist", bufs=1) as persist,
            tc.tile_pool(name="xbf", bufs=8) as xbf_pool,
            tc.tile_pool(name="xt", bufs=DC * NQ) as xt_pool,
            tc.tile_pool(name="et", bufs=20) as et_pool,
            tc.tile_pool(name="sacc", bufs=4) as sacc_pool,
            tc.tile_pool(name="epi", bufs=4) as epi_pool,
            tc.tile_pool(name="ps_tr", bufs=2, space="PSUM") as ps_tr,
            tc.tile_pool(name="ps_qkv", bufs=1, space="PSUM") as ps_qkv,
            tc.tile_pool(name="ps_sc", bufs=4, space="PSUM") as ps_sc,
            tc.tile_pool(name="ps_av", bufs=1, space="PSUM") as ps_av,
        ):
            # ---- constants ----
            ident_bf = const.tile([P, P], BF16, tag="ident_bf")
            make_identity(nc, ident_bf[:])
            ones_bf = const.tile([P, 1], BF16, tag="ones_bf")
            nc.gpsimd.memset(ones_bf[:], 1.0)
            one_f32 = const.tile([P, 1], F32, tag="one_f32")
            nc.gpsimd.memset(one_f32[:], 1.0)
            # causal masks for the 4 diagonal offsets: keep (q - j*128 - k) >= 0
            mask_t = []
            for j in range(SPB):
                mt = const.tile([P, QB], BF16, tag=f"mask{j}", name=f"mask{j}")
                nc.gpsimd.memset(mt[:], 1.0)
                nc.gpsimd.affine_select(
                    out=mt[:], in_=mt[:],
                    compare_op=mybir.AluOpType.is_ge,
                    fill=0.0, base=-j * P,
                    pattern=[[1, QB]], channel_multiplier=-1,
                )
                mask_t.append(mt)

            # weights, bf16, laid out [p=d%128, c=d//128, h]. Loaded f32 on
            # the HWDGE queue (keeps the SWDGE queue free for x) + DVE cast.
            w_sb = []
            for name in ("wq", "wk", "wv"):
                t = const.tile([P, DC, H], BF16, tag=f"w_{name}", name=f"w_{name}")
                w_sb.append(t)
            wq_sb, wk_sb, wv_sb = w_sb

            def emit_weight_loads():
                for t, wd in ((wq_sb, wq_d), (wk_sb, wk_d), (wv_sb, wv_d)):
                    stg = const.tile(
                        [P, DC, H], F32, tag=f"wstg_{t.tensor.name}",
                        name=f"wstg_{t.tensor.name}",
                    )
                    nc.sync.dma_start(
                        out=stg[:], in_=wd.ap().rearrange("(c p) h -> p c h", p=P)
                    )
                    nc.vector.tensor_copy(t[:], stg[:])

            q_sb = [persist.tile([P, QB], BF16, tag=f"q_sb{i}", name=f"q_sb{i}") for i in range(NQ)]
            k_sb = [persist.tile([P, QB], BF16, tag=f"k_sb{i}", name=f"k_sb{i}") for i in range(NQ)]
            vt_sb = [persist.tile([P, QB], BF16, tag=f"vt_sb{i}", name=f"vt_sb{i}") for i in range(NQ)]
            v_sb = persist.tile([P, SC, H], BF16, tag="v_sb")

            xt = [[None] * NQ for _ in range(DC)]
            for sr in range(NQ):
                for dc in range(DC):
                    xt[dc][sr] = xt_pool.tile([P, QB], BF16, tag="xt", name=f"xt_{dc}_{sr}")

            x_bf = [None] * SC

            def emit_loads(sr):
                for sj in range(SPB):
                    sc = sr * SPB + sj
                    x_bf[sc] = xbf_pool.tile([P, D], BF16, tag="xbf", name=f"xbf_{sc}")
                    nc.gpsimd.dma_start(
                        out=x_bf[sc][:], in_=x_d[sc * P : (sc + 1) * P, :]
                    )

            def emit_transposes(sr):
                # x^T for this s-range: per d-chunk, 4 transposes into one
                # grouped PSUM tile, then one wide copy into xt[dc][sr]
                for dc in range(DC):
                    tp = ps_tr.tile([P, SPB, P], BF16, tag="tr", name="tp")
                    for sj in range(SPB):
                        nc.tensor.transpose(
                            tp[:, sj, :],
                            x_bf[sr * SPB + sj][:, dc * P : (dc + 1) * P],
                            ident_bf[:],
                        )
                    nc.vector.tensor_copy(xt[dc][sr][:], tp[:])

            def emit_qkv(sr):
                for w_t, dst in ((wq_sb, q_sb[sr]), (wk_sb, k_sb[sr]), (wv_sb, vt_sb[sr])):
                    pr_ps = ps_qkv.tile([P, QB], F32, tag="qkv", name="pr_ps")
                    for dc in range(DC):
                        nc.tensor.matmul(
                            pr_ps[:], w_t[:, dc, :], xt[dc][sr][:],
                            start=(dc == 0), stop=(dc == DC - 1),
                        )
                    nc.scalar.copy(dst[:], pr_ps[:])
                # V chunks [k, h]: 4 transposes of V^T into one grouped bank
                tp = ps_tr.tile([P, SPB, P], BF16, tag="tr", name="tp_v")
                for sj in range(SPB):
                    nc.tensor.transpose(
                        tp[:, sj, :], vt_sb[sr][:, sj * P : (sj + 1) * P], ident_bf[:]
                    )
                nc.vector.tensor_copy(v_sb[:, sr * SPB : (sr + 1) * SPB, :], tp[:])

            def do_attention(qb):
                nkc = SPB * (qb + 1)
                av_ps = ps_av.tile([P, QB], F32, tag="av", name="av_ps")
                acc = [
                    sacc_pool.tile([P, QB], F32, tag=f"sacc{i}", name=f"acc{i}")
                    for i in range(2)
                ]
                e_tiles = [None] * nkc
                # diagonal chunks first: their exp->mask chain overlaps the
                # rest of the block instead of gating the AV tail
                order = list(range(qb * SPB, nkc)) + list(range(qb * SPB))

                def emit_av(pos):
                    kc = order[pos]
                    nc.tensor.matmul(
                        av_ps[:], v_sb[:, kc, :], e_tiles[kc][:],
                        start=(pos == 0), stop=(pos == nkc - 1),
                    )

                for pos, kc in enumerate(order):
                    sc_ps = ps_sc.tile([P, QB], F32, tag="sc", name="sc_ps")
                    nc.tensor.matmul(
                        sc_ps[:],
                        k_sb[kc // SPB][:, (kc % SPB) * P : (kc % SPB + 1) * P],
                        q_sb[qb][:],
                        start=True,
                        stop=True,
                    )
                    e_t = et_pool.tile([P, QB], BF16, tag="et", name="e_t")
                    e_tiles[kc] = e_t
                    nc.scalar.activation(
                        e_t[:], sc_ps[:], mybir.ActivationFunctionType.Exp,
                        scale=SCALE,
                    )
                    if kc >= qb * SPB:
                        # diagonal chunk: zero the k > q half (DVE, const mask)
                        nc.vector.tensor_mul(
                            e_t[:], e_t[:], mask_t[kc - qb * SPB][:]
                        )
                    # softmax denominators: two interleaved f32 chains on DVE
                    if pos < 2:
                        nc.vector.tensor_copy(acc[pos][:], e_t[:])
                    else:
                        a = acc[pos % 2]
                        nc.vector.tensor_add(a[:], a[:], e_t[:])
                    if pos >= AV_LAG:
                        emit_av(pos - AV_LAG)
                for pos in range(max(0, nkc - AV_LAG), nkc):
                    emit_av(pos)

                if nkc > 1:
                    nc.vector.tensor_add(acc[0][:], acc[0][:], acc[1][:])
                return av_ps, acc[0]

            def do_epilogue(qb, av_ps, acc):
                # sums: [1, QB] = ones.T @ acc (f32)
                sum_ps = ps_sc.tile([1, QB], F32, tag="sc", name="sum_ps")
                nc.tensor.matmul(
                    sum_ps[:], one_f32[:], acc[:], start=True, stop=True
                )
                sums_sb = epi_pool.tile([1, QB], F32, tag="sums_sb", name="sums_sb")
                nc.vector.tensor_copy(sums_sb[:], sum_ps[:])
                o_bf = epi_pool.tile([P, QB], BF16, tag="o_bf", name="o_bf")
                nc.vector.tensor_copy(o_bf[:], av_ps[:])
                for j in range(SPB):
                    # transpose sums [1,128] -> [128,1] via outer product
                    st_ps = ps_tr.tile([P, 1], F32, tag="tr", name="st_ps")
                    nc.tensor.matmul(
                        st_ps[:],
                        sums_sb[0:1, j * P : (j + 1) * P],
                        one_f32[0:1, :],
                        start=True,
                        stop=True,
                    )
                    rs_j = epi_pool.tile([P, 1], F32, tag="rs", name="rs_j")
                    nc.vector.reciprocal(rs_j[:], st_ps[:])
                    tr_ps = ps_tr.tile([P, P], BF16, tag="tr", name="tr_ps")
                    nc.tensor.transpose(
                        tr_ps[:], o_bf[:, j * P : (j + 1) * P], ident_bf[:]
                    )
                    out_sb = epi_pool.tile([P, H], F32, tag="out_sb", name="out_sb")
                    nc.vector.tensor_scalar_mul(out_sb[:], tr_ps[:], rs_j[:])
                    nc.sync.dma_start(
                        out=out_d[(qb * QB + j * P) : (qb * QB + (j + 1) * P), :],
                        in_=out_sb[:],
                    )

            # ---- main pipeline ----
            emit_weight_loads()
            emit_loads(0)
            emit_transposes(0)
            for sr in range(NQ):
                emit_qkv(sr)
                if sr + 1 < NQ:
                    emit_loads(sr + 1)
                av_ps, acc = do_attention(sr)
                if sr + 1 < NQ:
                    emit_transposes(sr + 1)
                do_epilogue(sr, av_ps, acc)

    nc.compile()
    return nc


def kernel(x, Wq, Wk, Wv):
    global _NC_CACHE
    if _NC_CACHE is None:
        _NC_CACHE = build()
    nc = _NC_CACHE
    x = np.ascontiguousarray(x, dtype=np.float32)
    in_maps = [
        {
            "x": np.ascontiguousarray(x[b]),
            "Wq": np.ascontiguousarray(Wq, dtype=np.float32),
            "Wk": np.ascontiguousarray(Wk, dtype=np.float32),
            "Wv": np.ascontiguousarray(Wv, dtype=np.float32),
        }
        for b in range(B)
    ]
    res = run_bass_kernel_spmd(nc, in_maps, core_ids=list(range(B)))
    return np.stack([res.results[b]["out"] for b in range(B)]).astype(np.float32)


# revision 12
# speedup vs baseline: 1.1390x; 1.1390x over previous
"""Causal single-head attention (B=8, S=2048, D=2048, H=128) on 8 TRN2 NeuronCores.

Strategy: data-parallel over batch — core b computes batch element b entirely
on-chip; no collectives. Per core:

  - x [S, D] f32 is cast-DMA'd to bf16 (SWDGE); x^T chunks [128d, 512s] are
    produced by PE transposes grouped 4-to-a-PSUM-bank with one wide DVE copy.
  - Q^T, K^T, V^T [h, s] via matmuls with Wq/Wk/Wv chunks stationary (bf16,
    1 cycle/row, moving free 512); V rechunked to [k, h] by PE transposes.
  - scores^T [k, q] = (lhsT=K^T chunk).T @ Q^T slice; exp on ScalarE with the
    1/sqrt(H) scale folded in; causal: upper-triangle chunks skipped entirely,
    diagonal chunks zeroed post-exp (gpsimd affine_select), and diagonal
    chunks are processed FIRST so the mask chain overlaps the block.
  - AV trails the scores/exp pipeline by AV_LAG chunks so exp latency never
    stalls the PE; softmax denominators accumulate on DVE as two interleaved
    f32 chains; per q-block one f32 ones-matmul + tiny outer-product
    transposes produce per-q reciprocals; epilogue transposes run in bf16.

All matmuls bf16 (f32 PSUM accumulation); rel err vs the f32 reference ~5e-3.
"""

import numpy as np

import concourse.bass as bass
import concourse.mybir as mybir
import concourse.tile as tile
from concourse import bacc
from concourse.bass_utils import run_bass_kernel_spmd
from concourse.masks import make_identity

B, S, D, H = 8, 2048, 2048, 128
P = 128
DC = D // P            # 16 d-chunks (contraction)
SC = S // P            # 16 s-chunks
QB = 512               # q-block (moving free dim)
NQ = S // QB           # 4 q-blocks
SPB = QB // P          # 4 s-chunks per q-block
SCALE = float(H) ** -0.5
AV_LAG = 4             # AV trails scores/exp by this many k-chunks

F32 = mybir.dt.float32
BF16 = mybir.dt.bfloat16

_NC_CACHE = None


def build():
    nc = bacc.Bacc(None, target_bir_lowering=False)

    x_d = nc.declare_dram_parameter("x", [S, D], F32, isOutput=False)
    wq_d = nc.declare_dram_parameter("Wq", [D, H], F32, isOutput=False)
    wk_d = nc.declare_dram_parameter("Wk", [D, H], F32, isOutput=False)
    wv_d = nc.declare_dram_parameter("Wv", [D, H], F32, isOutput=False)
    out_d = nc.declare_dram_parameter("out", [S, H], F32, isOutput=True)

    with tile.TileContext(nc) as tc:
        with (
            tc.tile_pool(name="const", bufs=1) as const,
            tc.tile_pool(name="persist", bufs=1) as persist,
            tc.tile_pool(name="xbf", bufs=8) as xbf_pool,
            tc.tile_pool(name="xt", bufs=DC * NQ) as xt_pool,
            tc.tile_pool(name="et", bufs=20) as et_pool,
            tc.tile_pool(name="sacc", bufs=4) as sacc_pool,
            tc.tile_pool(name="epi", bufs=4) as epi_pool,
            tc.tile_pool(name="ps_tr", bufs=2, space="PSUM") as ps_tr,
            tc.tile_pool(name="ps_qkv", bufs=1, space="PSUM") as ps_qkv,
            tc.tile_pool(name="ps_sc", bufs=4, space="PSUM") as ps_sc,
            tc.tile_pool(name="ps_av", bufs=1, space="PSUM") as ps_av,
        ):
            x_bf = [None] * SC

            def emit_loads(sr):
                for sj in range(SPB):
                    sc = sr * SPB + sj
                    x_bf[sc] = xbf_pool.tile([P, D], BF16, tag="xbf", name=f"xbf_{sc}")
                    nc.gpsimd.dma_start(
                        out=x_bf[sc][:], in_=x_d[sc * P : (sc + 1) * P, :]
                    )

            # x loads first: nothing else may delay the SWDGE stream
            emit_loads(0)

            # ---- constants ----
            ident_bf = const.tile([P, P], BF16, tag="ident_bf")
            make_identity(nc, ident_bf[:])
            one_f32 = const.tile([P, 1], F32, tag="one_f32")
            nc.gpsimd.memset(one_f32[:], 1.0)

            # weights, bf16, laid out [p=d%128, c=d//128, h]
            w_sb = []
            for name, wd in (("wq", wq_d), ("wk", wk_d), ("wv", wv_d)):
                t = const.tile([P, DC, H], BF16, tag=f"w_{name}", name=f"w_{name}")
                nc.gpsimd.dma_start(
                    out=t[:], in_=wd.ap().rearrange("(c p) h -> p c h", p=P)
                )
                w_sb.append(t)
            wq_sb, wk_sb, wv_sb = w_sb

            q_sb = [persist.tile([P, QB], BF16, tag=f"q_sb{i}", name=f"q_sb{i}") for i in range(NQ)]
            k_sb = [persist.tile([P, QB], BF16, tag=f"k_sb{i}", name=f"k_sb{i}") for i in range(NQ)]
            vt_sb = [persist.tile([P, QB], BF16, tag=f"vt_sb{i}", name=f"vt_sb{i}") for i in range(NQ)]
            v_sb = persist.tile([P, SC, H], BF16, tag="v_sb")

            xt = [[None] * NQ for _ in range(DC)]
            for sr in range(NQ):
                for dc in range(DC):
                    xt[dc][sr] = xt_pool.tile([P, QB], BF16, tag="xt", name=f"xt_{dc}_{sr}")

            def emit_transposes(sr):
                # x^T for this s-range: per d-chunk, 4 transposes into one
                # grouped PSUM tile, then one wide copy into xt[dc][sr]
                for dc in range(DC):
                    tp = ps_tr.tile([P, SPB, P], BF16, tag="tr", name="tp")
                    for sj in range(SPB):
                        nc.tensor.transpose(
                            tp[:, sj, :],
                            x_bf[sr * SPB + sj][:, dc * P : (dc + 1) * P],
                            ident_bf[:],
                        )
                    nc.vector.tensor_copy(xt[dc][sr][:], tp[:])

            def emit_qkv(sr):
                for w_t, dst in ((wq_sb, q_sb[sr]), (wk_sb, k_sb[sr]), (wv_sb, vt_sb[sr])):
                    pr_ps = ps_qkv.tile([P, QB], F32, tag="qkv", name="pr_ps")
                    for dc in range(DC):
                        nc.tensor.matmul(
                            pr_ps[:], w_t[:, dc, :], xt[dc][sr][:],
                            start=(dc == 0), stop=(dc == DC - 1),
                        )
                    nc.scalar.copy(dst[:], pr_ps[:])
                # V chunks [k, h]: 4 transposes of V^T into one grouped bank
                tp = ps_tr.tile([P, SPB, P], BF16, tag="tr", name="tp_v")
                for sj in range(SPB):
                    nc.tensor.transpose(
                        tp[:, sj, :], vt_sb[sr][:, sj * P : (sj + 1) * P], ident_bf[:]
                    )
                nc.vector.tensor_copy(v_sb[:, sr * SPB : (sr + 1) * SPB, :], tp[:])

            def do_attention(qb):
                nkc = SPB * (qb + 1)
                av_ps = ps_av.tile([P, QB], F32, tag="av", name="av_ps")
                acc = [
                    sacc_pool.tile([P, QB], F32, tag=f"sacc{i}", name=f"acc{i}")
                    for i in range(2)
                ]
                e_tiles = [None] * nkc
                # diagonal chunks first: their exp->mask chain overlaps the
                # rest of the block instead of gating the AV tail
                order = list(range(qb * SPB, nkc)) + list(range(qb * SPB))

                def emit_av(pos):
                    kc = order[pos]
                    nc.tensor.matmul(
                        av_ps[:], v_sb[:, kc, :], e_tiles[kc][:],
                        start=(pos == 0), stop=(pos == nkc - 1),
                    )

                for pos, kc in enumerate(order):
                    sc_ps = ps_sc.tile([P, QB], F32, tag="sc", name="sc_ps")
                    nc.tensor.matmul(
                        sc_ps[:],
                        k_sb[kc // SPB][:, (kc % SPB) * P : (kc % SPB + 1) * P],
                        q_sb[qb][:],
                        start=True,
                        stop=True,
                    )
                    e_t = et_pool.tile([P, QB], BF16, tag="et", name="e_t")
                    e_tiles[kc] = e_t
                    nc.scalar.activation(
                        e_t[:], sc_ps[:], mybir.ActivationFunctionType.Exp,
                        scale=SCALE,
                    )
                    if kc >= qb * SPB:
                        # diagonal chunk: keep (q - k) >= 0, else 0
                        nc.gpsimd.affine_select(
                            out=e_t[:],
                            in_=e_t[:],
                            compare_op=mybir.AluOpType.is_ge,
                            fill=0.0,
                            base=qb * QB - kc * P,
                            pattern=[[1, QB]],
                            channel_multiplier=-1,
                        )
                    # softmax denominators: two interleaved f32 chains on DVE
                    if pos < 2:
                        nc.vector.tensor_copy(acc[pos][:], e_t[:])
                    else:
                        a = acc[pos % 2]
                        nc.vector.tensor_add(a[:], a[:], e_t[:])
                    if pos >= AV_LAG:
                        emit_av(pos - AV_LAG)
                for pos in range(max(0, nkc - AV_LAG), nkc):
                    emit_av(pos)

                if nkc > 1:
                    nc.vector.tensor_add(acc[0][:], acc[0][:], acc[1][:])
                return av_ps, acc[0]

            def do_epilogue(qb, av_ps, acc):
                # sums: [1, QB] = ones.T @ acc (f32)
                sum_ps = ps_sc.tile([1, QB], F32, tag="sc", name="sum_ps")
                nc.tensor.matmul(
                    sum_ps[:], one_f32[:], acc[:], start=True, stop=True
                )
                sums_sb = epi_pool.tile([1, QB], F32, tag="sums_sb", name="sums_sb")
                nc.vector.tensor_copy(sums_sb[:], sum_ps[:])
                o_bf = epi_pool.tile([P, QB], BF16, tag="o_bf", name="o_bf")
                nc.vector.tensor_copy(o_bf[:], av_ps[:])
                for j in range(SPB):
                    # transpose sums [1,128] -> [128,1] via outer product
                    st_ps = ps_tr.tile([P, 1], F32, tag="tr", name="st_ps")
                    nc.tensor.matmul(
                        st_ps[:],
                        sums_sb[0:1, j * P : (j + 1) * P],
                        one_f32[0:1, :],
                        start=True,
                        stop=True,
                    )
                    rs_j = epi_pool.tile([P, 1], F32, tag="rs", name="rs_j")
                    nc.vector.reciprocal(rs_j[:], st_ps[:])
                    tr_ps = ps_tr.tile([P, P], BF16, tag="tr", name="tr_ps")
                    nc.tensor.transpose(
                        tr_ps[:], o_bf[:, j * P : (j + 1) * P], ident_bf[:]
                    )
                    out_sb = epi_pool.tile([P, H], F32, tag="out_sb", name="out_sb")
                    nc.vector.tensor_scalar_mul(out_sb[:], tr_ps[:], rs_j[:])
                    nc.sync.dma_start(
                        out=out_d[(qb * QB + j * P) : (qb * QB + (j + 1) * P), :],
                        in_=out_sb[:],
                    )

            # ---- main pipeline ----
            emit_transposes(0)
            for sr in range(NQ):
                emit_qkv(sr)
                if sr + 1 < NQ:
                    emit_loads(sr + 1)
                av_ps, acc = do_attention(sr)
                if sr + 1 < NQ:
                    emit_transposes(sr + 1)
                do_epilogue(sr, av_ps, acc)

    nc.compile()
    return nc


def kernel(x, Wq, Wk, Wv):
    global _NC_CACHE
    if _NC_CACHE is None:
        _NC_CACHE = build()
    nc = _NC_CACHE
    x = np.ascontiguousarray(x, dtype=np.float32)
    in_maps = [
        {
            "x": np.ascontiguousarray(x[b]),
            "Wq": np.ascontiguousarray(Wq, dtype=np.float32),
            "Wk": np.ascontiguousarray(Wk, dtype=np.float32),
            "Wv": np.ascontiguousarray(Wv, dtype=np.float32),
        }
        for b in range(B)
    ]
    res = run_bass_kernel_spmd(nc, in_maps, core_ids=list(range(B)))
    return np.stack([res.results[b]["out"] for b in range(B)]).astype(np.float32)
